# revision 1
# baseline (speedup 1.0000x reference)
"""MoE-routing actor kernel for 8 Trainium2 NeuronCores.

Strategy (pure data parallel, expert-sorted, bf16 matmul, int8 output):
  - Host: fc1 trunk + relu on BLAS; rows dealt per-expert round-robin to the
    8 cores (shared SPMD graph); per-expert capacities trimmed so each core
    is exactly 32 supers of 1024 rows (the few overflow rows are computed
    exactly on host, as are kept mask columns beyond the 128 PSUM width).
  - Output is int8 with per-(expert, column) scales estimated from a 32k-row
    sample (margin 1.4); 1/scale is folded into the bf16 expert weights and
    bias (ones-row trick), so the device casts PSUM f32 -> int8 directly and
    the host multiplies scales back during the gather. Store traffic halves
    vs bf16; unmasked rel err ~1.2e-2 (gate 2e-2).
  - Device (raw bacc): per 1024-row super, expert matmuls alternate PE
    partition strips 0/64 (concurrent sub-arrays, ~0.5 cyc/row). PSUM->int8
    casts alternate DVE/ACT per super -- the binding resource: PSUM has one
    read port per engine, so 32 supers / 2 engines ~ 18us paces everything.
  - Loads: weff (both strip replicas in one 99-partition DMA) + first
    2-super group on sync; group-0 high half + group 1 on scalar (after its
    ACT-table load); remaining ramped groups (3,4,5,6,6,4 supers) on the
    gpsimd SWDGE queue. First matmul waits only weff + group-0-low; the h1
    strip gates on group-0-high, hiding ~2.4us DMA completion receipts.
  - Stores: 256KB int8 super-pairs on sync with NO final completion wait --
    the walrus block-exit drain fences the ring during the (fixed, ~8us)
    semaphore-reset epilogue, hiding the last store's receipt latency.
"""

import os
import sys

sys.path.insert(0, "/opt/trn_rl_repo")

import numpy as np
import ml_dtypes

BF16 = ml_dtypes.bfloat16

B = 262144
NCORES = 8
J = 16
M = 12
H = 34
HP = H + 1  # fc1 output + ones row for bias folding
S_DIM = 32  # state dim
A = J * J  # 256 action logits
NEG = np.float32(-1.0e9)
SUPER = 1024  # rows per compute chunk
HALF = 512  # PSUM-bank / matmul free-dim granule
NP = 4  # psum ring depth (supers)

_BUILD_CACHE: dict = {}
LAST_RESULT = None  # BassKernelResults of the most recent run (for profiling)


def _make_runs(caps, R):
    """Per 512-row half-chunk, the (expert, row0, row1) runs covering it."""
    offs = np.concatenate([[0], np.cumsum(caps)])
    assert offs[-1] == R
    runs = [[] for _ in range(R // HALF)]
    for m in range(len(caps)):
        lo, hi = int(offs[m]), int(offs[m + 1])
        if lo >= hi:
            continue
        for g in range(lo // HALF, (hi - 1) // HALF + 1):
            a = max(lo, g * HALF)
            b = min(hi, (g + 1) * HALF)
            if a < b:
                runs[g].append((m, a, b))
    return runs


GROUPS = (2, 2, 3, 4, 5, 6, 6, 4)  # ramped super counts per load group


def _build(R: int, caps: tuple, Adev: int):
    """Raw-bacc device graph: manual semaphores, static SBUF allocation.

    R must be a multiple of 1024 (whole supers); sum(GROUPS) supers total.
    Input loads use a ramped group schedule: group 0 (weff + first supers)
    is split across the sync and vector HWDGE rings so descriptor
    generation parallelizes and the first matmul starts ~10us in; group 1
    goes on the scalar ring (after its ACT-table load); the rest stream on
    the gpsimd SWDGE queue whose per-DMA descriptor-gen (~0.7us) would
    otherwise pace the whole pipeline.
    """
    from concourse import bacc, mybir

    n_half = R // HALF
    n_super = n_half // 2
    assert n_super == sum(GROUPS)
    runs = _make_runs(list(caps), R)
    f32 = mybir.dt.float32
    bf16 = mybir.dt.bfloat16
    i8 = mybir.dt.int8
    nc = bacc.Bacc("TRN2", target_bir_lowering=False, debug=False)

    n_grp = len(GROUPS)
    gbase = [sum(GROUPS[:g]) for g in range(n_grp)]  # first super of group g
    cbase = [b * HALF for b in gbase]  # first xa column of group g
    sup2grp = {}
    for g in range(n_grp):
        for j in range(GROUPS[g]):
            sup2grp[gbase[g] + j] = (g, j)
    n_pair = n_super // 2

    xat_ds = [
        nc.declare_dram_parameter(f"xat{g}", [2, HP, GROUPS[g] * HALF], bf16,
                                  isOutput=False)
        for g in range(n_grp)
    ]
    # both partition replicas (rows 0:35 and 64:99) in ONE dma: the host
    # duplicates the content so the first matmul waits on a single sem
    weff_d = nc.declare_dram_parameter("weff", [64 + HP, M * Adev], bf16,
                                       isOutput=False)
    out_d = nc.declare_dram_parameter(
        "out", [n_pair, Adev, 2 * SUPER], i8, isOutput=True
    )

    xa = nc.alloc_sbuf_tensor("xa_sb", [64 + HP, n_super * HALF], bf16)
    weff = nc.alloc_sbuf_tensor("weff_sb", [64 + HP, M * Adev], bf16)
    otb = nc.alloc_sbuf_tensor("ot_sb", [Adev, n_super * SUPER], i8)
    ots = [otb[:, s * SUPER : (s + 1) * SUPER] for s in range(n_super)]
    psos = [nc.alloc_psum_tensor(f"pso{k}", [Adev, SUPER], f32) for k in range(NP)]

    NSX = 4  # rotating input-load sems
    NSQ = 4  # rotating store sems
    sem_w = nc.alloc_semaphore("sem_w")
    sem_g0a = nc.alloc_semaphore("sem_g0a")
    sem_g0b = nc.alloc_semaphore("sem_g0b")
    sem_x = [nc.alloc_semaphore(f"sem_x{k}") for k in range(NSX)]
    sem_mm = nc.alloc_semaphore("sem_mm")
    sem_cv = nc.alloc_semaphore("sem_cv")
    sem_ca = nc.alloc_semaphore("sem_ca")
    sem_oe = [nc.alloc_semaphore(f"sem_oe{k}") for k in range(NSQ)]

    def gslice(g):
        return slice(cbase[g], cbase[g] + GROUPS[g] * HALF)

    with nc.Block() as block:

        @block.gpsimd
        def _(g):
            for gi in range(2, n_grp):
                sx = sem_x[(gi - 1) % NSX]
                if gi - 1 >= NSX:
                    g.wait_ge(sx, 32 * ((gi - 1) // NSX))
                g.dma_start(xa[0:HP, gslice(gi)], xat_ds[gi][0]).then_inc(sx, 16)
                g.dma_start(xa[64 : 64 + HP, gslice(gi)], xat_ds[gi][1]).then_inc(
                    sx, 16
                )

        # cast-engine assignment: DVE takes even supers, ACT takes odd supers
        dve_set = [sc for sc in range(0, n_super, 2)]
        act_set = [sc for sc in range(1, n_super, 2)]
        dve_rank = {sc: i + 1 for i, sc in enumerate(dve_set)}
        act_rank = {sc: i + 1 for i, sc in enumerate(act_set)}

        def wait_cast_done(eng, k):
            if k in dve_rank:
                eng.wait_ge(sem_cv, dve_rank[k])
            else:
                eng.wait_ge(sem_ca, act_rank[k])

        @block.tensor
        def _(t):
            t.wait_ge(sem_w, 16)
            t.wait_ge(sem_g0a, 16)  # group-0 low half only; h1 gated below
            for sc in range(n_super):
                gi, j = sup2grp[sc]
                if j == 0 and gi > 0:
                    t.wait_ge(sem_x[(gi - 1) % NSX], 32 * ((gi - 1) // NSX + 1))
                if sc >= NP:
                    wait_cast_done(t, sc - NP)
                pso = psos[sc % NP]
                mms = []
                for h in range(2):
                    base = 0 if h == 0 else 64
                    if sc == 0 and h == 1:
                        t.wait_ge(sem_g0b, 16)
                    for (m, a, b) in runs[sc * 2 + h]:
                        c0 = a - sc * SUPER
                        c1 = b - sc * SUPER
                        xcol = cbase[gi] + j * HALF
                        mms.append(
                            t.matmul(
                                pso[:, c0:c1],
                                weff[base : base + HP, m * Adev : (m + 1) * Adev],
                                xa[
                                    base : base + HP,
                                    xcol + c0 - h * HALF : xcol + c1 - h * HALF,
                                ],
                                start=True,
                                stop=True,
                            )
                        )
                mms[-1].then_inc(sem_mm, 1)

        @block.vector
        def _(v):
            for sc in dve_set:
                v.wait_ge(sem_mm, sc + 1)
                v.tensor_copy(ots[sc][:, :], psos[sc % NP][:, :]).then_inc(sem_cv, 1)

        @block.scalar
        def _(s):
            # group-0 high half + group 1 on the scalar HWDGE ring (first op
            # lands after the preamble ACT-table load, before gpsimd's SWDGE)
            s.dma_start(xa[64 : 64 + HP, gslice(0)], xat_ds[0][1]).then_inc(
                sem_g0b, 16
            )
            s.dma_start(xa[0:HP, gslice(1)], xat_ds[1][0]).then_inc(sem_x[0], 16)
            s.dma_start(xa[64 : 64 + HP, gslice(1)], xat_ds[1][1]).then_inc(
                sem_x[0], 16
            )
            for sc in act_set:
                s.wait_ge(sem_mm, sc + 1)
                s.copy(ots[sc][:, :], psos[sc % NP][:, :]).then_inc(sem_ca, 1)

        @block.sync
        def _(sy):
            sy.dma_start(weff[0 : 64 + HP, :], weff_d[:]).then_inc(sem_w, 16)
            sy.dma_start(xa[0:HP, gslice(0)], xat_ds[0][0]).then_inc(sem_g0a, 16)
            for p in range(n_pair):
                wait_cast_done(sy, 2 * p)
                wait_cast_done(sy, 2 * p + 1)
                so = sem_oe[p % NSQ]
                if p >= NSQ:
                    sy.wait_ge(so, 16 * (p // NSQ))
                sy.dma_start(
                    out_d[p][:], otb[:, 2 * p * SUPER : (2 * p + 2) * SUPER]
                ).then_inc(so, 16)
            # no final completion wait: the block-exit drain fences the HWDGE
            # ring, so in-flight stores land before the NEFF retires; this
            # hides the last store's ~2us completion receipt in the epilogue

    nc.compile()
    return nc


def kernel(states, epoch_idx, W1, b1, Wout, bout, mask):
    global LAST_RESULT
    from concourse.bass_utils import run_bass_kernel_spmd

    states = np.asarray(states, dtype=np.float32)
    epoch_idx = np.asarray(epoch_idx, dtype=np.int32)
    W1 = np.asarray(W1, dtype=np.float32)
    b1 = np.asarray(b1, dtype=np.float32)
    Wout = np.asarray(Wout, dtype=np.float32)
    bout = np.asarray(bout, dtype=np.float32)
    mask = np.asarray(mask, dtype=np.int32)

    keep = mask.reshape(A) != 0
    kept_cols = np.nonzero(keep)[0]
    Ak = int(len(kept_cols))
    if Ak == 0:
        return np.full((B, J, J), NEG, np.float32)
    Adev = min(Ak, 128)
    dev_cols = kept_cols[:Adev]
    rem_cols = kept_cols[Adev:]

    # --- shared trunk on host (tiny: ~0.6 GFLOP BLAS) ---
    x = np.maximum(states @ W1.T + b1[None, :], 0.0)  # [B, H] f32

    # --- route rows: per expert, deal round-robin across cores ---
    core_idx = [[None] * M for _ in range(NCORES)]
    for m in range(M):
        idx_m = np.nonzero(epoch_idx == m)[0]
        for i in range(NCORES):
            core_idx[i][m] = idx_m[i::NCORES]
    cnt = [[len(core_idx[i][m]) for m in range(M)] for i in range(NCORES)]
    # shared per-expert row capacity across cores (row-granular)
    caps = [max(cnt[i][m] for i in range(NCORES)) for m in range(M)]
    # R is fixed at exactly sum(GROUPS) supers; trim caps down to fit (the
    # handful of overflow rows are computed exactly on host) or pad up.
    R = sum(GROUPS) * SUPER
    excess = sum(caps) - R
    while excess > 0:
        m_big = max(range(M), key=lambda m: caps[m])
        d = min(excess, max(1, excess // M))
        caps[m_big] -= d
        excess -= d
    if excess < 0:
        caps[-1] += -excess
    caps = tuple(caps)
    offs = np.concatenate([[0], np.cumsum(caps)])
    ncap = [[min(cnt[i][m], caps[m]) for m in range(M)] for i in range(NCORES)]

    # --- int8 output scales: per-(expert, column), estimated from a sample ---
    # The device stores logits/s as int8; weff/bout are pre-divided by s so
    # the PSUM value is already scaled, and the host multiplies back after
    # the gather.  Sample max * margin covers the population max; rare
    # overflows only clip (or wrap) a handful of elements of 33M.
    SAMP = 32768
    MARGIN = 1.4
    rng = np.random.default_rng(12345)
    samp = rng.choice(B, SAMP, replace=False)
    scale = np.empty((M, Adev), np.float32)
    for m in range(M):
        rows_s = samp[epoch_idx[samp] == m]
        sl = x[rows_s] @ Wout[m][dev_cols].T + bout[m][dev_cols]
        scale[m] = np.abs(sl).max(axis=0) * (MARGIN / 127.0)

    # --- effective expert weights (device columns only; bout in ones row;
    # content duplicated at partition bases 0 and 64 for the PE strip trick) ---
    weff = np.zeros((64 + HP, M * Adev), np.float32)
    for m in range(M):
        weff[:H, m * Adev : (m + 1) * Adev] = Wout[m][dev_cols].T / scale[m][None, :]
        weff[H, m * Adev : (m + 1) * Adev] = bout[m][dev_cols] / scale[m]
    weff[64 : 64 + HP] = weff[:HP]
    weff_bf = weff.astype(BF16)

    # --- pack per-core transposed activations (bf16, ramped groups) ---
    n_super = R // SUPER
    gbase = [sum(GROUPS[:g]) for g in range(len(GROUPS))]
    in_maps = []
    for i in range(NCORES):
        packed = np.zeros((R, HP), np.float32)
        packed[:, H] = 1.0  # ones row for bias folding
        for m in range(M):
            r0 = int(offs[m])
            packed[r0 : r0 + ncap[i][m], :H] = x[core_idx[i][m][: caps[m]]]
        pv = packed.astype(BF16).reshape(n_super, 2, HALF, HP)
        imap = {"weff": weff_bf}
        for g, gs in enumerate(GROUPS):
            imap[f"xat{g}"] = np.ascontiguousarray(
                pv[gbase[g] : gbase[g] + gs]
                .transpose(1, 3, 0, 2)
                .reshape(2, HP, gs * HALF)
            )
        in_maps.append(imap)

    key = (R, caps, Adev)
    nc = _BUILD_CACHE.get(key)
    if nc is None:
        nc = _build(R, caps, Adev)
        _BUILD_CACHE[key] = nc

    # retry: rare transient NRT_EXEC_UNIT_UNRECOVERABLE on fresh NEFFs
    last_err = None
    for _attempt in range(3):
        try:
            res = run_bass_kernel_spmd(nc, in_maps, core_ids=list(range(NCORES)))
            break
        except Exception as e:  # noqa: BLE001
            last_err = e
    else:
        raise last_err
    LAST_RESULT = res

    # --- unpack: [n_pair, Adev, 2048] int8 -> rows [R, Adev], dequantize ---
    out_kept = np.zeros((B, Adev), np.float32)
    for i in range(NCORES):
        oc = np.asarray(res.results[i]["out"])
        rows = oc.transpose(0, 2, 1).reshape(-1, Adev)[:R]
        for m in range(M):
            r0 = int(offs[m])
            out_kept[core_idx[i][m][: caps[m]]] = (
                rows[r0 : r0 + ncap[i][m]].astype(np.float32) * scale[m][None, :]
            )

    out_full = np.full((B, A), NEG, np.float32)
    out_full[:, dev_cols] = out_kept

    # --- host remainder: kept columns beyond the device's 128, plus the
    # few per-core cap-overflow rows (exact f32) ---
    for m in range(M):
        rows_m = np.nonzero(epoch_idx == m)[0]
        if len(rem_cols):
            out_full[rows_m[:, None], rem_cols[None, :]] = (
                x[rows_m] @ Wout[m][rem_cols].T + bout[m][rem_cols][None, :]
            )
        ov = np.concatenate(
            [core_idx[i][m][caps[m] :] for i in range(NCORES)]
        ).astype(np.int64)
        if len(ov):
            out_full[ov[:, None], dev_cols[None, :]] = (
                x[ov] @ Wout[m][dev_cols].T + bout[m][dev_cols][None, :]
            )

    return out_full.reshape(B, J, J)

